# revision 33
# baseline (speedup 1.0000x reference)
"""Trainium2 Bass kernel for nn_DeepONetCfCDecoder.

Strategy (8 NeuronCores, data-parallel over queries, time-banded):
  * Host: searchsorted -> per-query time-bucket idx; stable-sort queries by
    idx; split into 8 equal rank-chunks (one per core).  Each core gets a
    contiguous band of h_states buckets plus its queries packed into tiles of
    128 that each cover a window of <= G consecutive buckets.
  * Host also precomputes the full 72-dim trunk input per query (fourier
    positional encoding, dt time embedding, component embedding) and ships it
    pre-transposed, together with the bucket one-hot rows used for the
    additive attention mask, in one [85,128] block per tile.
  * Device: per core, build an interleaved K^T/V table for its band with two
    matmul families (weights pre-folded on host: W_k = btok_w@bk_w/sqrt(H),
    W_v = btok_w@bv_w), then per tile: trunk matmul + tanh-based SiLU,
    layernorm (E[x2]-E[x]2 form, means free via accum_out), q projection,
    block-masked attention reading the table directly with dynamic offsets,
    context MLP, and the rank-basis contraction.  All scalar-engine
    activations live in the single `exp_and_others` table set (tanh/exp/
    identity/copy) so no ACT_TABLE_LOAD thrash occurs in steady state.
  * rel_bias of the reference is structurally zero (LayerNorm over a
    singleton axis -> 0; rb1 = rb2 = 0) and constant-per-row score offsets
    cancel in softmax, so the whole relative-position branch is dropped.
    Other structurally-zero biases are elided via build flags computed from
    the actual inputs (generic paths retained behind the flags).
"""

import os
import sys

sys.path.insert(0, "/opt/trn_rl_repo")

import numpy as np
import ml_dtypes

import concourse.bass as bass
import concourse.mybir as mybir
import concourse.tile as tile
import bass_rust as _bass_rust
from concourse.bass_utils import run_bass_kernel_spmd

BF16 = ml_dtypes.bfloat16
F32 = mybir.dt.float32
I32 = mybir.dt.int32
BF = mybir.dt.bfloat16
AF = mybir.ActivationFunctionType
ALU = mybir.AluOpType

N, K, T, D = 8192, 64, 512, 256
H, RANK, DTDIM, FH, L = 256, 256, 32, 8, 1.0
NCORES = 8
G = 12          # bucket slots per tile window (must be even)
P = 128         # queries per tile
NEG = -30000.0  # additive mask value


def _pack(t_q, sensor_time):
    """Sort queries by bucket, chunk to cores, pack 128-query tiles."""
    idx = np.clip(np.searchsorted(sensor_time, t_q, side="right") - 1, 0, T - 1)
    order = np.argsort(idx, kind="stable")
    per_core = N // NCORES
    raw = []
    maxB = maxTPC = 0
    for i in range(NCORES):
        sel = order[i * per_core:(i + 1) * per_core]
        bidx = idx[sel]
        lo = int(bidx[0])
        Bc = int(bidx[-1]) - lo + 1
        tiles = []
        pos = 0
        while pos < len(sel):
            b0 = int(bidx[pos]) - lo
            s = b0 - (b0 % 2)
            take, g = [], []
            while pos < len(sel) and len(take) < P and int(bidx[pos]) - lo < s + G:
                take.append(sel[pos])
                g.append(int(bidx[pos]) - lo - s)
                pos += 1
            nreal = len(take)
            while len(take) < P:
                take.append(take[-1])
                g.append(g[-1])
            tiles.append([s, np.array(take), np.array(g, np.int64), nreal])
        raw.append((lo, Bc, tiles))
        maxB = max(maxB, Bc)
        maxTPC = max(maxTPC, len(tiles))
    B = max(maxB, G)
    B = (B + 15) // 16 * 16       # even units + 512-divisible DMA halves
    TPC = maxTPC
    cores = []
    for lo, Bc, tiles in raw:
        fixed = []
        for s, q, g, nr in tiles:
            s2 = min(s, B - G)
            fixed.append((s2, q, g + (s - s2), nr))
        while len(fixed) < TPC:
            fixed.append((0, fixed[-1][1], np.zeros(P, np.int64), 0))
        cores.append((lo, fixed))
    return cores, B, TPC, idx


def _build(B, TPC, flags):
    B64 = B * 64
    NU = B // 2                   # 512-wide interleaved units
    has_tob = flags["has_tob"]
    has_bpb = flags["has_bpb"]
    has_cb1 = flags["has_cb1"]
    has_cv = flags["has_cv"]
    has_cbias = flags["has_cbias"]
    has_bqb = flags["has_bqb"]
    nc = bass.Bass()

    def inp(name, shape, dt=BF):
        return nc.declare_dram_parameter(name, list(shape), dt, isOutput=False)

    ht_d = inp("ht", [128, 2 * B64])
    wk_d = inp("wk", [128, 512])
    wv_d = inp("wv", [128, 512])
    trunkw_d = inp("trunkw", [73, 256])
    bq_d = inp("bqw", [128, 512])
    cw1_d = inp("cw1w", [128, 512])
    cw2_d = inp("cw2w", [128, 512])
    tow_d = inp("tow", [128, 1536])
    bpw_d = inp("bpw", [128, 1536])
    rowb_d = inp("rowb", [1, 1536])
    expander_d = inp("expander", [12, 768])
    ppb_d = inp("ppb", [128, 8], F32)
    ident_d = inp("ident", [128, 128])
    ones_d = inp("ones", [1, 128])
    cvrow_d = inp("cvrow", [1, 256], F32)
    onesf_d = inp("onesf", [1, 128], F32)
    qfall_d = inp("qfall", [73, TPC * 128])
    ohall_d = inp("ohall", [12, TPC * 128])
    ohs_d = inp("ohs", [128, 4 * TPC], F32)
    moff_d = inp("moff", [1, TPC], I32)
    out_d = nc.declare_dram_parameter("out", [128, TPC], F32, isOutput=True)
    KDEBUG = bool(os.environ.get("KDEBUG"))
    if KDEBUG:
        dbg_d = nc.declare_dram_parameter("dbg", [128, 2048], F32, isOutput=True)

    with tile.TileContext(nc) as tc:
        with (
            tc.tile_pool(name="const", bufs=1) as cp,
            tc.tile_pool(name="work", bufs=6) as wp,
            tc.tile_pool(name="cslab", bufs=5) as csp,
            tc.tile_pool(name="psum", bufs=2, space="PSUM") as pp,
        ):
            # ---------------- startup DMAs ----------------
            # sync queue: small early-needed tensors, then the first half of
            # the h_states band; scalar queue: the rest of the weights, then
            # the second half of the band.  Tiles are bucket-sorted, so early
            # tiles touch the first half of the K/V table.
            id_bf = cp.tile([128, 128], BF, tag="id_bf")
            nc.sync.dma_start(id_bf[:], ident_d[:])
            trunkw_sb = cp.tile([73, 256], BF, tag="trunkw")
            nc.sync.dma_start(trunkw_sb[:], trunkw_d[:])
            qfall = cp.tile([73, TPC * 128], BF, tag="qfall")
            nc.sync.dma_start(qfall[:], qfall_d[:])
            ohall = cp.tile([12, TPC * 128], BF, tag="ohall")
            nc.sync.dma_start(ohall[:], ohall_d[:])
            moff_sb = cp.tile([1, TPC], I32, tag="moff")
            nc.sync.dma_start(moff_sb[:], moff_d[:])
            ppb_sb = cp.tile([128, 8], F32, tag="ppb")
            nc.sync.dma_start(ppb_sb[:], ppb_d[:])
            ohs_sb = cp.tile([128, 4 * TPC], F32, tag="ohs")
            nc.sync.dma_start(ohs_sb[:], ohs_d[:])
            wk_sb = cp.tile([128, 512], BF, tag="wk")
            nc.scalar.dma_start(wk_sb[:], wk_d[:])
            wv_sb = cp.tile([128, 512], BF, tag="wv")
            nc.scalar.dma_start(wv_sb[:], wv_d[:])
            expander_sb = cp.tile([12, 768], BF, tag="expander")
            nc.scalar.dma_start(expander_sb[:], expander_d[:])
            tow_sb = cp.tile([128, 1536], BF, tag="tow")
            nc.scalar.dma_start(tow_sb[:], tow_d[:])
            bq_sb = cp.tile([128, 512], BF, tag="bq")
            nc.scalar.dma_start(bq_sb[:], bq_d[:])
            cw1_sb = cp.tile([128, 512], BF, tag="cw1")
            nc.scalar.dma_start(cw1_sb[:], cw1_d[:])
            cw2_sb = cp.tile([128, 512], BF, tag="cw2")
            nc.scalar.dma_start(cw2_sb[:], cw2_d[:])
            bpw_sb = cp.tile([128, 1536], BF, tag="bpw")
            nc.scalar.dma_start(bpw_sb[:], bpw_d[:])
            if has_tob or has_bpb:
                rowb_sb = cp.tile([1, 1536], BF, tag="rowb")
                nc.scalar.dma_start(rowb_sb[:], rowb_d[:])
                ones1 = cp.tile([1, 128], BF, tag="ones1")
                nc.scalar.dma_start(ones1[:], ones_d[:])
            if has_cv:
                onesf = cp.tile([1, 128], F32, tag="onesf")
                nc.scalar.dma_start(onesf[:], onesf_d[:])
                cvrow_sb = cp.tile([1, 256], F32, tag="cvrow")
                nc.scalar.dma_start(cvrow_sb[:], cvrow_d[:])
                cv_rep = cp.tile([128, 256], F32, tag="cv_rep")
                psb = pp.tile([128, 512], F32, tag="scps")
                nc.tensor.matmul(psb[:, 0:256], onesf[:], cvrow_sb[0:1, 0:256],
                                 start=True, stop=True)
                nc.vector.tensor_copy(cv_rep[:], psb[:, 0:256])

            # h_states band split: first j-half on sync (fast), second on
            # scalar after the weights.
            htA = cp.tile([128, B64], BF, tag="htA")   # d-chunk 0
            htB = cp.tile([128, B64], BF, tag="htB")   # d-chunk 1
            HB2 = B64 // 2
            nc.sync.dma_start(htA[:, 0:HB2], ht_d[:, 0:HB2])
            nc.sync.dma_start(htB[:, 0:HB2], ht_d[:, B64:B64 + HB2])
            nc.scalar.dma_start(htA[:, HB2:B64], ht_d[:, HB2:B64])
            nc.scalar.dma_start(htB[:, HB2:B64], ht_d[:, B64 + HB2:2 * B64])
            hts = (htA, htB)

            # ---------------- phase 1: K^T and V tables ----------------
            # interleaved table: per 2-bucket unit u (=128 j-rows):
            #   cols [512u,512u+128) = K^T chunk0, +128..256 = K^T chunk1,
            #   +256..512 = V rows of unit u.  Dynamic windows read directly.
            ctab = cp.tile([128, NU * 512], BF, tag="ctab")
            ctab_v = ctab[:].rearrange("p (u blk) -> p u blk", blk=512)
            eng = [nc.vector, nc.scalar]
            ei = 0
            for f0 in range(0, B64, 512):
                for ch in range(2):
                    ps = pp.tile([128, 512], F32, tag="scps")
                    for dch in range(2):
                        nc.tensor.matmul(
                            ps[:, 0:512],
                            wk_sb[:, (dch * 2 + ch) * 128:(dch * 2 + ch + 1) * 128],
                            hts[dch][:, f0:f0 + 512],
                            start=(dch == 0),
                            stop=(dch == 1),
                        )
                    dst = ctab_v[:, f0 // 128:f0 // 128 + 4, ch * 128:(ch + 1) * 128]
                    psv = ps[:, 0:512].rearrange("p (u blk) -> p u blk", blk=128)
                    if ei % 2 == 0:
                        nc.vector.tensor_copy(dst, psv)
                    else:
                        nc.scalar.activation(dst, psv, AF.Copy)
                    ei += 1
            for jp in range(B64 // 256):
                ps = pp.tile([128, 512], F32, tag="scps")
                for half in range(2):
                    js = 2 * jp + half
                    for dch in range(2):
                        nc.tensor.matmul(
                            ps[:, half * 256:(half + 1) * 256],
                            hts[dch][:, js * 128:(js + 1) * 128],
                            wv_sb[:, dch * 256:(dch + 1) * 256],
                            start=(dch == 0),
                            stop=(dch == 1),
                        )
                dst = ctab_v[:, 2 * jp:2 * jp + 2, 256:512]
                psv2 = ps[:].rearrange("p (u blk) -> p u blk", blk=256)
                if ei % 2 == 0:
                    nc.vector.tensor_copy(dst, psv2)
                else:
                    nc.scalar.activation(dst, psv2, AF.Copy)
                ei += 1

            out_sb = cp.tile([128, TPC], F32, tag="out_sb")
            if KDEBUG:
                dbg_sb = cp.tile([128, 2048], F32, tag="dbg_sb")

            # ---------------- phase 2: per-tile pipeline ----------------
            def rstd_chain(vsum, vsumsq, tag):
                """mean + 1/sqrt(var) from sum & sumsq (fast rsqrt).

                eps elided (var is bounded well away from 0 for this net).
                """
                m = wp.tile([128, 1], F32, tag=tag + "_m")
                nc.vector.tensor_scalar(m[:], vsum[:], 1.0 / 256, None, ALU.mult)
                m2 = wp.tile([128, 1], F32, tag=tag + "_m2")
                nc.vector.tensor_tensor(m2[:], m[:], m[:], ALU.mult)
                vv = wp.tile([128, 1], F32, tag=tag + "_vv")
                nc.vector.scalar_tensor_tensor(
                    vv[:], vsumsq[:], 1.0 / 256, m2[:], ALU.mult, ALU.subtract)
                t0 = wp.tile([128, 1], I32, tag=tag + "_t0")
                nc.vector.tensor_scalar(t0[:], vv[:].bitcast(I32), 1, None,
                                        ALU.arith_shift_right)
                y0i = wp.tile([128, 1], I32, tag=tag + "_y0")
                nc.vector.tensor_scalar(y0i[:], t0[:], 0x5F3759DF, -1,
                                        ALU.subtract, ALU.mult)
                y0 = y0i[:].bitcast(F32)
                t1n = wp.tile([128, 1], F32, tag=tag + "_t1n")
                nc.vector.tensor_tensor(t1n[:], y0, y0, ALU.mult)
                t2 = wp.tile([128, 1], F32, tag=tag + "_t2")
                nc.vector.scalar_tensor_tensor(
                    t2[:], t1n[:], 0.5, vv[:], ALU.mult, ALU.mult)
                t3 = wp.tile([128, 1], F32, tag=tag + "_t3")
                nc.vector.tensor_scalar(t3[:], t2[:], 1.5, -1.0,
                                        ALU.subtract, ALU.mult)
                rstd = wp.tile([128, 1], F32, tag=tag + "_r")
                nc.vector.tensor_tensor(rstd[:], y0, t3[:], ALU.mult)
                negmr = wp.tile([128, 1], F32, tag=tag + "_nmr")
                nc.vector.scalar_tensor_tensor(
                    negmr[:], m[:], -1.0, rstd[:], ALU.mult, ALU.mult)
                return rstd, negmr

            # ---- 6-stage skewed software pipeline over tiles ----
            # Per-engine queues execute in order, so emitting all of tile m
            # before tile m+1 leaves engines idle during m's serial segments.
            # Skewing stages across tiles keeps every engine fed.
            st = [dict() for _ in range(TPC)]

            def s0(m):  # window copy + trunk + silu + LN1
                t = st[m]
                qf = qfall[:, m * 128:(m + 1) * 128]
                coff = nc.values_load(
                    moff_sb[0:1, m:m + 1],
                    engines=[mybir.EngineType.DVE],
                    min_val=0, max_val=(NU - G // 2) * 512,
                    skip_runtime_bounds_check=True,
                )
                cslab = csp.tile([128, 3072], BF, tag="cslab")
                nc.vector.tensor_copy(cslab[:], ctab[:, bass.ds(coff, 3072)])
                t["cslab_v"] = cslab[:].rearrange("p (u blk) -> p u blk", blk=512)
                fq_ps = pp.tile([128, 256], F32, tag="early")
                nc.tensor.matmul(fq_ps[:], qf, trunkw_sb[:],
                                 start=True, stop=True)
                tsig = wp.tile([128, 256], BF, tag="tsig")
                nc.scalar.activation(tsig[:], fq_ps[:], AF.Tanh, scale=0.5)
                fqh = wp.tile([128, 256], BF, tag="fqh")
                fsum = wp.tile([128, 1], F32, tag="fsum")
                nc.vector.scalar_tensor_tensor(
                    fqh[:], tsig[:], 1.0, fq_ps[:], ALU.add, ALU.mult,
                    accum_out=fsum[:],
                )
                sqs = wp.tile([128, 256], F32, tag="sqs")
                fsumsq = wp.tile([128, 1], F32, tag="fsumsq")
                nc.scalar.activation(sqs[:], fqh[:], AF.Square,
                                     accum_out=fsumsq[:])
                r1, nmr1 = rstd_chain(fsum, fsumsq, "ln1")
                lnf = wp.tile([128, 256], BF, tag="lnf")
                nc.scalar.activation(lnf[:], fqh[:], AF.Identity,
                                     scale=r1[:], bias=nmr1[:])
                t["fqh"], t["lnf"] = fqh, lnf
                if KDEBUG and m == 0:
                    nc.vector.tensor_copy(dbg_sb[:, 0:256], fqh[:])
                    nc.vector.tensor_copy(dbg_sb[:, 256:512], lnf[:])

            def s1(m):  # transposes + trunk basis + q projection
                t = st[m]
                fqh, lnf = t.pop("fqh"), t.pop("lnf")
                tpFG = pp.tile([128, 512], BF, tag="tp")
                for ich in range(2):
                    nc.tensor.transpose(
                        tpFG[:, ich * 128:(ich + 1) * 128],
                        fqh[:, ich * 128:(ich + 1) * 128], id_bf[:])
                    nc.tensor.transpose(
                        tpFG[:, 256 + ich * 128:256 + (ich + 1) * 128],
                        lnf[:, ich * 128:(ich + 1) * 128], id_bf[:])
                fTsT = wp.tile([128, 256], BF, tag="fTsT")
                nc.vector.tensor_copy(fTsT[:], tpFG[:, 0:256])
                lnT = wp.tile([128, 256], BF, tag="lnT")
                nc.scalar.activation(lnT[:], tpFG[:, 256:512], AF.Copy)
                tb_sb = wp.tile([128, 768], BF, tag="tb_sb")
                for f0, fw, tg in ((0, 512, "scps"), (512, 256, "early")):
                    tbp = pp.tile([128, fw], F32, tag=tg)
                    for hch in range(2):
                        nc.tensor.matmul(
                            tbp[:],
                            fTsT[:, hch * 128:(hch + 1) * 128],
                            tow_sb[:, hch * 768 + f0:hch * 768 + f0 + fw],
                            start=(hch == 0),
                            stop=(hch == 1) and not has_tob,
                        )
                    if has_tob:
                        nc.tensor.matmul(
                            tbp[:], ones1[:],
                            rowb_sb[0:1, 768 + f0:768 + f0 + fw],
                            start=False, stop=True,
                        )
                    nc.scalar.activation(tb_sb[:, f0:f0 + fw], tbp[:], AF.Copy)
                qT_ps = pp.tile([128, 256], F32, tag="early")
                for ich in range(2):
                    for hch in range(2):
                        nc.tensor.matmul(
                            qT_ps[:, ich * 128:(ich + 1) * 128],
                            bq_sb[:, (hch * 2 + ich) * 128:(hch * 2 + ich + 1) * 128],
                            lnT[:, hch * 128:(hch + 1) * 128],
                            start=(hch == 0), stop=(hch == 1),
                        )
                qT = wp.tile([128, 256], BF, tag="qT")
                if has_bqb:
                    for ich in range(2):
                        nc.scalar.activation(
                            qT[:, ich * 128:(ich + 1) * 128],
                            qT_ps[:, ich * 128:(ich + 1) * 128],
                            AF.Identity, bias=ppb_sb[:, 2 + ich:3 + ich],
                        )
                else:
                    nc.scalar.activation(qT[:], qT_ps[:], AF.Copy)
                t["tb_sb"], t["qT"] = tb_sb, qT
                if KDEBUG and m == 0:
                    nc.vector.tensor_copy(dbg_sb[:, 1024:1280], qT[:])

            def s2(m):  # scores + mask + exp + transpose of attention weights
                t = st[m]
                qT = t.pop("qT")
                cslab_v = t["cslab_v"]
                expm = wp.tile([128, 768], BF, tag="expm")
                den2 = wp.tile([128, 2], F32, tag="den2")
                for i, (f0, fw, u0, nu, tg) in enumerate(
                        ((0, 512, 0, 4, "scps"), (512, 256, 4, 2, "late"))):
                    scp = pp.tile([128, fw], F32, tag=tg)
                    for dch in range(2):
                        nc.tensor.matmul(
                            scp[:],
                            qT[:, dch * 128:(dch + 1) * 128],
                            cslab_v[:, u0:u0 + nu, dch * 128:(dch + 1) * 128],
                            start=(dch == 0), stop=False,
                        )
                    nc.tensor.matmul(
                        scp[:],
                        ohall[:, m * 128:(m + 1) * 128],
                        expander_sb[:, f0:f0 + fw],
                        start=False, stop=True,
                    )
                    nc.scalar.activation(
                        expm[:, f0:f0 + fw], scp[:], AF.Exp,
                        accum_out=den2[:, i:i + 1],
                    )
                recip = wp.tile([128, 1], F32, tag="recip")
                nc.vector.tensor_tensor(recip[:], den2[:, 0:1], den2[:, 1:2],
                                        ALU.add)
                nc.vector.reciprocal(recip[:], recip[:])
                tpC = pp.tile([128, 768], BF, tag="tp")
                for j in range(6):
                    nc.tensor.transpose(
                        tpC[:, j * 128:(j + 1) * 128],
                        expm[:, j * 128:(j + 1) * 128], id_bf[:])
                expT = wp.tile([128, 768], BF, tag="expT")
                nc.vector.tensor_copy(expT[:], tpC[:])
                t["expT"], t["recip"] = expT, recip
                if KDEBUG and m == 0:
                    nc.vector.tensor_copy(dbg_sb[:, 1280:2048], expm[:])

            def s3(m):  # context + LN2 stats
                t = st[m]
                expT, recip = t.pop("expT"), t.pop("recip")
                cslab_v = t.pop("cslab_v")
                ctx_ps = pp.tile([128, 256], F32, tag="late")
                for j in range(6):
                    nc.tensor.matmul(
                        ctx_ps[:],
                        expT[:, j * 128:(j + 1) * 128],
                        cslab_v[:, j, 256:512],
                        start=(j == 0), stop=(j == 5),
                    )
                ctx = wp.tile([128, 256], F32, tag="ctx")
                csum = wp.tile([128, 1], F32, tag="csum")
                if has_cv:
                    nc.vector.scalar_tensor_tensor(
                        ctx[:], ctx_ps[:], recip[:], cv_rep[:],
                        ALU.mult, ALU.add, accum_out=csum[:],
                    )
                else:
                    nc.scalar.activation(ctx[:], ctx_ps[:], AF.Identity,
                                         scale=recip[:], accum_out=csum[:])
                sqc = wp.tile([128, 256], F32, tag="sqc")
                csumsq = wp.tile([128, 1], F32, tag="csumsq")
                nc.scalar.activation(sqc[:], ctx[:], AF.Square,
                                     accum_out=csumsq[:])
                r2, nmr2 = rstd_chain(csum, csumsq, "ln2")
                lnc = wp.tile([128, 256], BF, tag="lnc")
                nc.scalar.activation(lnc[:], ctx[:], AF.Identity,
                                     scale=r2[:], bias=nmr2[:])
                t["ctx"], t["lnc"] = ctx, lnc
                if KDEBUG and m == 0:
                    nc.vector.tensor_copy(dbg_sb[:, 512:768], ctx[:])

            def s4(m):  # context MLP
                t = st[m]
                ctx, lnc = t.pop("ctx"), t.pop("lnc")
                tpD = pp.tile([128, 256], BF, tag="tp")
                for ich in range(2):
                    nc.tensor.transpose(
                        tpD[:, ich * 128:(ich + 1) * 128],
                        lnc[:, ich * 128:(ich + 1) * 128], id_bf[:])
                lncT = wp.tile([128, 256], BF, tag="lncT")
                nc.vector.tensor_copy(lncT[:], tpD[:])
                h1_ps = pp.tile([128, 256], F32, tag="late")
                for ich in range(2):
                    for hch in range(2):
                        nc.tensor.matmul(
                            h1_ps[:, ich * 128:(ich + 1) * 128],
                            cw1_sb[:, (hch * 2 + ich) * 128:(hch * 2 + ich + 1) * 128],
                            lncT[:, hch * 128:(hch + 1) * 128],
                            start=(hch == 0), stop=(hch == 1),
                        )
                tsig2 = wp.tile([128, 256], BF, tag="tsig2")
                if has_cb1:
                    for ich in range(2):
                        nc.scalar.activation(
                            tsig2[:, ich * 128:(ich + 1) * 128],
                            h1_ps[:, ich * 128:(ich + 1) * 128],
                            AF.Tanh, scale=0.5, bias=ppb_sb[:, 4 + ich:5 + ich],
                        )
                    xb = wp.tile([128, 256], F32, tag="xb")
                    for ich in range(2):
                        nc.vector.tensor_scalar(
                            xb[:, ich * 128:(ich + 1) * 128],
                            h1_ps[:, ich * 128:(ich + 1) * 128],
                            ppb_sb[:, 6 + ich:7 + ich], None, ALU.add)
                    h1T = wp.tile([128, 256], BF, tag="h1T")
                    nc.vector.scalar_tensor_tensor(
                        h1T[:], tsig2[:], 1.0, xb[:], ALU.add, ALU.mult)
                else:
                    nc.scalar.activation(tsig2[:], h1_ps[:], AF.Tanh, scale=0.5)
                    h1T = wp.tile([128, 256], BF, tag="h1T")
                    nc.vector.scalar_tensor_tensor(
                        h1T[:], tsig2[:], 1.0, h1_ps[:], ALU.add, ALU.mult)
                mlp_ps = pp.tile([128, 256], F32, tag="late")
                for ich in range(2):
                    nc.tensor.matmul(
                        mlp_ps[:],
                        h1T[:, ich * 128:(ich + 1) * 128],
                        cw2_sb[:, ich * 256:(ich + 1) * 256],
                        start=(ich == 0), stop=(ich == 1),
                    )
                ctx3 = wp.tile([128, 256], BF, tag="ctx3")
                nc.vector.tensor_tensor(ctx3[:], mlp_ps[:], ctx[:], ALU.add)
                tpE = pp.tile([128, 256], BF, tag="tp")
                for ich in range(2):
                    nc.tensor.transpose(
                        tpE[:, ich * 128:(ich + 1) * 128],
                        ctx3[:, ich * 128:(ich + 1) * 128], id_bf[:])
                ctx3T = wp.tile([128, 256], BF, tag="ctx3T")
                nc.vector.tensor_copy(ctx3T[:], tpE[:])
                t["ctx3T"] = ctx3T
                if KDEBUG and m == 0:
                    nc.vector.tensor_copy(dbg_sb[:, 768:1024], ctx3[:])

            def s5(m):  # branch basis + rank contraction + output column
                t = st[m]
                ctx3T = t.pop("ctx3T")
                tb_sb = t.pop("tb_sb")
                s3t = wp.tile([128, 3], F32, tag="s3")
                scratch = wp.tile([128, 256], F32, tag="scratch")
                bps_l = [pp.tile([128, 256], F32, tag="late",
                                 name=f"bps{_c}_{m}")[:] for _c in range(3)]
                for hch in range(2):
                    for comp in range(3):
                        nc.tensor.matmul(
                            bps_l[comp],
                            ctx3T[:, hch * 128:(hch + 1) * 128],
                            bpw_sb[:, hch * 768 + comp * 256:hch * 768 + (comp + 1) * 256],
                            start=(hch == 0),
                            stop=(hch == 1) and not has_bpb,
                        )
                for comp in range(3):
                    if has_bpb:
                        nc.tensor.matmul(
                            bps_l[comp], ones1[:],
                            rowb_sb[0:1, comp * 256:(comp + 1) * 256],
                            start=False, stop=True,
                        )
                    nc.vector.scalar_tensor_tensor(
                        scratch[:], bps_l[comp], 1.0,
                        tb_sb[:, comp * 256:(comp + 1) * 256],
                        ALU.mult, ALU.mult, accum_out=s3t[:, comp:comp + 1],
                    )
                scr3 = wp.tile([128, 3], F32, tag="scr3")
                nc.vector.scalar_tensor_tensor(
                    scr3[:], s3t[:], 1.0, ohs_sb[:, 4 * m:4 * m + 3],
                    ALU.mult, ALU.mult, accum_out=out_sb[:, m:m + 1],
                )
                if has_cbias:
                    nc.vector.tensor_tensor(
                        out_sb[:, m:m + 1], out_sb[:, m:m + 1],
                        ohs_sb[:, 4 * m + 3:4 * m + 4], ALU.add)

            stages = [s0, s1, s2, s3, s4, s5]
            NS = len(stages)
            for k in range(TPC + NS - 1):
                for s in range(NS):
                    mm = k - s
                    if 0 <= mm < TPC:
                        stages[s](mm)

            nc.sync.dma_start(out_d[:], out_sb[:])
            if KDEBUG:
                nc.sync.dma_start(dbg_d[:], dbg_sb[:])
    # split multi-waits: HW allows at most one sync wait per instruction
    _bass_rust.move_matmul_waits_to_ldweights(nc.m)
    _bass_rust.generate_event_semaphores(nc)
    return nc


def _prepare(inputs):
    ins = {k: np.asarray(v) for k, v in inputs.items()}
    t_q = ins["t_q"].astype(np.float32)
    st = ins["sensor_time"].astype(np.float32)
    xy = ins["xy"].astype(np.float32)
    c = ins["c"].astype(np.int64)
    h = ins["h_states"].astype(np.float32)

    cores, B, TPC, idx = _pack(t_q, st)
    B64 = B * 64

    # ---- host-side parameter folds ----
    W_k = (ins["btok_w"] @ ins["bk_w"]) / np.float32(np.sqrt(H))
    W_v = ins["btok_w"] @ ins["bv_w"]
    cv = ins["btok_b"] @ ins["bv_w"] + ins["bv_b"]
    bq_w_eff = ins["bn_g"][:, None] * ins["bq_w"]
    bq_b_eff = ins["bn_b"] @ ins["bq_w"] + ins["bq_b"]
    cw1_eff = ins["cln_g"][:, None] * ins["cw1"]
    cb1_eff = ins["cln_b"] @ ins["cw1"] + ins["cb1"]
    bp_b_eff = ins["cb2"] @ ins["bp_w"] + ins["bp_b"]
    temp = float(np.exp(ins["log_temp"][0]))

    flags = dict(
        has_tob=bool(np.any(ins["to_b"])),
        has_bpb=bool(np.any(bp_b_eff)),
        has_cb1=bool(np.any(cb1_eff)),
        has_cv=bool(np.any(cv)),
        has_cbias=bool(np.any(ins["comp_bias"])),
        has_bqb=bool(np.any(bq_b_eff)),
    )

    def chunk2(w):  # [256, X] -> [128, 2*X]  (col = dch*X + x)
        x = w.shape[1]
        return np.ascontiguousarray(
            w.reshape(2, 128, x).transpose(1, 0, 2).reshape(128, 2 * x)
        ).astype(BF16)

    def chunk22(w):  # [256, 256] -> [128, 512]  (col = (dch*2+ich)*128 + i)
        return np.ascontiguousarray(
            w.reshape(2, 128, 2, 128).transpose(1, 0, 2, 3).reshape(128, 512)
        ).astype(BF16)

    wk_h = chunk22(W_k)
    bq_h = chunk22(bq_w_eff)
    cw1_h = chunk22(cw1_eff)
    wv_h = chunk2(W_v)
    cw2_h = chunk2(ins["cw2"] * 0.5)          # tanh-silu fold
    tow_h = chunk2(ins["to_w"] * 0.5)         # tanh-silu fold
    bpw_h = chunk2(ins["bp_w"])
    trunkw_h = np.concatenate(
        [ins["trunk_in_w"], ins["trunk_in_b"][None, :]], axis=0).astype(BF16)
    rowb_h = np.concatenate([bp_b_eff, ins["to_b"]]).astype(BF16)[None, :]
    ppb_h = np.ascontiguousarray(np.stack([
        np.zeros(128, np.float32), np.zeros(128, np.float32),
        bq_b_eff[0:128], bq_b_eff[128:256],
        0.5 * cb1_eff[0:128], 0.5 * cb1_eff[128:256],
        cb1_eff[0:128], cb1_eff[128:256],
    ]).T).astype(np.float32)
    cvrow_h = cv.astype(np.float32)[None, :]
    expander_h = np.full((12, 768), NEG, np.float32)
    for s in range(12):
        expander_h[s, s * 64:(s + 1) * 64] = 0.0
    expander_h = expander_h.astype(BF16)

    # ---- host trunk-input features for all queries ----
    harm = np.arange(1, FH + 1, dtype=np.float32)
    ang = 2.0 * np.pi * xy[:, :, None] * harm / L            # [N,2,FH]
    pos_enc = np.concatenate(
        [np.sin(ang), np.cos(ang)], axis=-1).reshape(N, 4 * FH)
    dt = np.maximum(t_q - st[idx], 0.0)                      # [N]
    time_e = dt[:, None] @ ins["time_proj_w"].astype(np.float32) \
        + ins["time_proj_b"].astype(np.float32)
    emb_c = ins["comp_emb"].astype(np.float32)[c]
    feat = np.concatenate([pos_enc, time_e, emb_c], axis=1).astype(np.float32)
    ohsc = np.zeros((N, 4), np.float32)
    ohsc[np.arange(N), c] = temp * ins["comp_scale"].astype(np.float32)[c]
    ohsc[:, 3] = ins["comp_bias"].astype(np.float32)[c]

    shared = dict(
        wk=wk_h, wv=wv_h, trunkw=trunkw_h, bqw=bq_h, cw1w=cw1_h, cw2w=cw2_h,
        tow=tow_h, bpw=bpw_h, rowb=rowb_h, expander=expander_h, ppb=ppb_h,
        cvrow=cvrow_h, ident=np.eye(128, dtype=BF16),
        ones=np.ones((1, 128), BF16), onesf=np.ones((1, 128), np.float32),
    )

    in_maps = []
    slotmaps = []
    for lo, tiles in cores:
        hb = np.zeros((B, K, D), np.float32)
        nb = min(B, T - lo)
        hb[:nb] = h[lo:lo + nb]
        ht_h = np.ascontiguousarray(
            hb.reshape(B64, D).T.reshape(2, 128, B64).transpose(1, 0, 2).reshape(128, 2 * B64)
        ).astype(BF16)
        qfall_h = np.zeros((73, TPC * 128), np.float32)
        ohall_h = np.zeros((12, TPC * 128), np.float32)
        ohs_h = np.zeros((128, 4 * TPC), np.float32)
        moff_h = np.zeros((1, TPC), np.int32)
        smap = np.full((TPC, 128), -1, np.int64)
        for mth, (s, qsel, g, nreal) in enumerate(tiles):
            qfall_h[0:72, mth * 128:(mth + 1) * 128] = feat[qsel].T
            qfall_h[72, mth * 128:(mth + 1) * 128] = 1.0
            ohall_h[:, mth * 128:(mth + 1) * 128] = (
                g[None, :] == np.arange(12)[:, None]).astype(np.float32)
            ohs_h[:, 4 * mth:4 * (mth + 1)] = ohsc[qsel]
            moff_h[0, mth] = 256 * s
            smap[mth, :nreal] = qsel[:nreal]
        in_maps.append(dict(ht=ht_h, qfall=qfall_h.astype(BF16),
                            ohall=ohall_h.astype(BF16), ohs=ohs_h,
                            moff=moff_h, **shared))
        slotmaps.append(smap)
    return in_maps, slotmaps, B, TPC, flags


_last_run = None


def kernel(**inputs):
    global _last_run
    in_maps, slotmaps, B, TPC, flags = _prepare(inputs)
    nc = _build(B, TPC, flags)
    _last_run = run_bass_kernel_spmd(nc, in_maps, list(range(NCORES)))
    results = _last_run.results

    out_full = np.zeros(N, np.float32)
    for ci in range(NCORES):
        o = np.asarray(results[ci]["out"])          # [128, TPC]
        sm = slotmaps[ci]                           # [TPC, 128]
        valid = sm >= 0
        out_full[sm[valid]] = o.T[valid]
    return out_full
